# revision 52
# baseline (speedup 1.0000x reference)
"""Multi-head attention (B=2, S=2048, D=1024, H=16) on 8 NeuronCores.

Sharding: core c -> batch c//4, head-group c%4 (4 heads, 256 proj dims).
Per-core Bass/Tile kernel computes Q/K/V projections, transposed-scores
attention (k on partitions, softmax without max-subtraction), and a
partial output projection (row-parallel Wo). Host sums the 4 partials
per batch and adds bo.

v2 over the original baseline (which ran the attention phase at the
HAM-throttled 1.2GHz PE clock):
- x / Wq / Wk / Wv shipped bf16: halves input DMA, enables FWL.
- Scores psum is one [128, 2048] f32 tile; exp is issued per-h2
  [128, 1024] so scores(kt+1, h2) only WARs exp(kt, h2) and the ACT
  engine stays back-to-back saturated while scores matmuls hide.
- Zero-stationary junk matmuls (+= 0 * x into the live O.T psum
  accumulation) fill the PE stall windows: always-ready real PE
  streaming, keeps the HAM clock gate at 2.4GHz, costs no psum bank.
- Softmax denominator reciprocal via DMA-transpose to [128, 16]
  (was a 6.5us single-lane [1, 1024] DVE reciprocal, 52us total).
"""

import sys

sys.path.insert(0, "/opt/trn_rl_repo")

from contextlib import ExitStack

import numpy as np

import concourse.bacc as bacc
import concourse.mybir as mybir
import concourse.tile as tile
from concourse.bass_utils import run_bass_kernel_spmd

B = 2
S = 2048
D = 1024
H = 16
HD = 64
HPC = 4          # heads per core
DPC = HPC * HD   # 256 projection dims per core
NCORES = 8
SCALE = 8.0      # sqrt(HD)

F32 = mybir.dt.float32
F32R = mybir.dt.float32r
BF16 = mybir.dt.bfloat16
ADT = BF16   # attention operand dtype (qt/kt/v/pt)

DCH = D // 128   # 8 contraction chunks of 128
QT = S // 128    # 16 q-tiles / k-tiles of 128
QCN = 2          # attention q-chunks of 1024
QCW = 1024

JUNK_PER_KT = 2   # zero-weight filler matmuls per kt (HAM warmth)
BOUNDARY_JUNK = 12  # filler burst at group boundaries without outproj
PROJ_JUNK = 2
FINAL_JUNK = 40     # filler per projection d-chunk (DMA-paced phase)


def build_nc():
    nc = bacc.Bacc("TRN2", target_bir_lowering=False, debug=False, num_devices=NCORES)

    xq = nc.dram_tensor("xq_t", [D, S], BF16, kind="ExternalInput")
    xk = nc.dram_tensor("xk_t", [D, S], BF16, kind="ExternalInput")
    xv = nc.dram_tensor("xv_t", [D, S], BF16, kind="ExternalInput")
    wq = nc.dram_tensor("wq_t", [D, DPC], BF16, kind="ExternalInput")
    wk = nc.dram_tensor("wk_t", [D, DPC], BF16, kind="ExternalInput")
    wv = nc.dram_tensor("wv_t", [D, DPC], BF16, kind="ExternalInput")
    wo = nc.dram_tensor("wo_t", [DPC, D], F32R, kind="ExternalInput")
    bq = nc.dram_tensor("bq", [DPC, 1], F32, kind="ExternalInput")
    bk = nc.dram_tensor("bk", [DPC, 1], F32, kind="ExternalInput")
    bv = nc.dram_tensor("bv", [DPC, 1], F32, kind="ExternalInput")
    ident = nc.dram_tensor("ident", [128, 128], F32R, kind="ExternalInput")
    y = nc.dram_tensor("y", [S, D], BF16, kind="ExternalOutput")

    with tile.TileContext(nc) as tc, ExitStack() as ctx:
        const = ctx.enter_context(tc.tile_pool(name="const", bufs=1))
        xin = ctx.enter_context(tc.tile_pool(name="xin", bufs=5))
        qkv = ctx.enter_context(tc.tile_pool(name="qkv", bufs=1))
        ptp = ctx.enter_context(tc.tile_pool(name="ptp", bufs=2))
        nrm = ctx.enter_context(tc.tile_pool(name="nrm", bufs=2))
        yp = ctx.enter_context(tc.tile_pool(name="yp", bufs=3))

        # ---- constants / weights ----
        # tiny dummy exp first: preloads the ACT exp table off the
        # critical path
        dmy = const.tile([1, 16], F32, tag="dmy")
        nc.vector.memset(dmy[:], 0.0)
        dmy2 = const.tile([1, 16], F32, tag="dmy2")
        nc.scalar.activation(dmy2[:], dmy[:], mybir.ActivationFunctionType.Exp)

        id_sb = const.tile([128, 128], F32R, tag="id")

        # memset can't target f32r; stage in f32 and round via DVE copy
        onesv32 = const.tile([128, HPC], F32, tag="onesv32")
        nc.vector.memset(onesv32[:], 1.0)
        onesv = const.tile([128, HPC], ADT, tag="onesv")
        nc.vector.tensor_copy(onesv[:], onesv32[:])
        # zero-weight tile for HAM-filler matmuls (adds 0 to live psum)
        zw32 = const.tile([128, HD + 1], F32, tag="zw32")
        nc.vector.memset(zw32[:], 0.0)
        zw = const.tile([128, HD + 1], ADT, tag="zw")
        nc.vector.tensor_copy(zw[:], zw32[:])

        # weights: V first (V-projection runs first), then Q, K
        wv_sb = [const.tile([128, DPC], BF16, tag=f"wv{d}", name=f"wv{d}") for d in range(DCH)]
        wq_sb = [const.tile([128, DPC], BF16, tag=f"wq{d}", name=f"wq{d}") for d in range(DCH)]
        wk_sb = [const.tile([128, DPC], BF16, tag=f"wk{d}", name=f"wk{d}") for d in range(DCH)]
        bq_sb = [const.tile([128, 1], F32, tag=f"bq{hp}", name=f"bq{hp}") for hp in range(2)]
        bk_sb = [const.tile([128, 1], F32, tag=f"bk{hp}", name=f"bk{hp}") for hp in range(2)]
        bv_sb = [const.tile([128, 1], F32, tag=f"bv{hp}", name=f"bv{hp}") for hp in range(2)]
        # biases on the (idle) gpsimd SWDGE queue; weight matrices are
        # interleaved with the x chunk loads inside the proj loop so the
        # early hwdge queue bandwidth goes to the critical-path tensors
        for hp in range(2):
            nc.gpsimd.dma_start(bv_sb[hp][:], bv[hp * 128:(hp + 1) * 128, :])
            nc.gpsimd.dma_start(bq_sb[hp][:], bq[hp * 128:(hp + 1) * 128, :])
            nc.gpsimd.dma_start(bk_sb[hp][:], bk[hp * 128:(hp + 1) * 128, :])
        wo_sb = [const.tile([128, D], F32R, tag=f"wo{g}", name=f"wo{g}") for g in range(2)]
        w_dram = {"v": wv, "q": wq, "k": wk}
        w_sbs = {"v": wv_sb, "q": wq_sb, "k": wk_sb}

        # ---- V tiles (128, 4*65) with ones column, filled by PE transpose
        # of a V.T projection ----
        v_sb = [qkv.tile([128, HPC * (HD + 1)], ADT, tag=f"v{st}", name=f"v{st}") for st in range(QT)]
        for st in range(QT):
            v4 = v_sb[st][:].rearrange("p (h w) -> p h w", h=HPC)
            nc.vector.tensor_copy(
                v4[:, :, HD:HD + 1],
                onesv[:].rearrange("p (a b) -> p a b", b=1),
            )
        vt_sb = [qkv.tile([128, S], F32R, tag=f"vt{hp}", name=f"vtt{hp}") for hp in range(2)]

        # ---- Q.T / K.T projections: (d'=hp*128 partitions, s free) ----
        qt_sb = [qkv.tile([128, S], ADT, tag=f"qt{hp}", name=f"qtt{hp}") for hp in range(2)]
        kt_sb = [qkv.tile([128, S], ADT, tag=f"kt{hp}", name=f"ktt{hp}") for hp in range(2)]
        with tc.tile_pool(name="ps_p", bufs=1, space="PSUM") as ps_p:
            nc.scalar.dma_start(id_sb[:], ident[:])
            for g in range(2):
                nc.scalar.dma_start(wo_sb[g][:], wo[g * 128:(g + 1) * 128, :])
            tr_q = [(hp, st) for hp in range(2) for st in range(QT)]

            def emit_transposes(n):
                # V.T -> V transposes, interleaved into the q/k
                # projection passes (fills their DMA slack; the vt
                # source is complete once the v pass evacuated)
                for _ in range(min(n, len(tr_q))):
                    hp, st = tr_q.pop(0)
                    tp = ps_p.tile([128, 128], F32R, tag=f"pp{st % 8}",
                                   name=f"tp{hp}{st}")
                    nc.tensor.transpose(
                        tp[:],
                        vt_sb[hp][:, st * 128:(st + 1) * 128],
                        id_sb[:],
                    )
                    v4 = v_sb[st][:].rearrange("p (h w) -> p h w", h=HPC)
                    nc.vector.tensor_copy(
                        v4[:, 2 * hp:2 * hp + 2, 0:HD],
                        tp[:].rearrange("p (h w) -> p h w", h=2),
                    )

            for which, xin_dram, w_sb, b_sb, dst in (
                ("v", xv, wv_sb, bv_sb, vt_sb),
                ("q", xq, wq_sb, bq_sb, qt_sb),
                ("k", xk, wk_sb, bk_sb, kt_sb),
            ):
                accs = {}
                for hp in range(2):
                    for pc in range(4):
                        accs[(hp, pc)] = ps_p.tile([128, 512], F32, tag=f"pp{hp * 4 + pc}", name=f"pp_{which}{hp}{pc}")
                for d in range(DCH):
                    # weight chunk just ahead of its x chunk; x halves
                    # split across both hwdge queues for 2x bandwidth
                    nc.scalar.dma_start(w_sbs[which][d][:],
                                        w_dram[which][d * 128:(d + 1) * 128, :])
                    xt = xin.tile([128, S], BF16, tag="x")
                    if which == "v" and d < 2:
                        # quarter-grain first chunks: the d=0 matmuls
                        # start after 128KB instead of 512KB
                        for pc in range(4):
                            eng = nc.sync if pc % 2 == 0 else nc.scalar
                            eng.dma_start(
                                xt[:, pc * 512:(pc + 1) * 512],
                                xin_dram[d * 128:(d + 1) * 128,
                                         pc * 512:(pc + 1) * 512])
                    else:
                        nc.sync.dma_start(
                            xt[:, 0:S // 2],
                            xin_dram[d * 128:(d + 1) * 128, 0:S // 2])
                        nc.scalar.dma_start(
                            xt[:, S // 2:S],
                            xin_dram[d * 128:(d + 1) * 128, S // 2:S])
                    for hp in range(2):
                        for pc in range(4):
                            nc.tensor.matmul(
                                accs[(hp, pc)][:],
                                w_sb[d][:, hp * 128:(hp + 1) * 128],
                                xt[:, pc * 512:(pc + 1) * 512],
                                start=(d == 0), stop=(d == DCH - 1),
                            )
                    if which != "v" and not (which == "q" and d < 2):
                        emit_transposes(3)
                    if d >= 1:
                        # zero-weight filler vs the x-chunk DMA pacing:
                        # keeps the HAM clock warm through the proj phase
                        for i in range(PROJ_JUNK):
                            nc.tensor.matmul(
                                accs[(i % 2, i // 2 % 4)][0:HD + 1, 0:256],
                                zw[:],
                                w_sb[d][:],
                                start=False, stop=False,
                                skip_group_check=True,
                            )
                for hp in range(2):
                    for pc in range(4):
                        nc.vector.tensor_scalar_add(
                            dst[hp][:, pc * 512:(pc + 1) * 512],
                            accs[(hp, pc)][:],
                            b_sb[hp][:],
                        )
            emit_transposes(len(tr_q))

        # ---- attention + normalization, head-pairs packed on PE rows ----
        otn_sb = [qkv.tile([128, S], F32R, tag=f"otn{j}", name=f"otn{j}") for j in range(2)]
        with tc.tile_pool(name="ps_s", bufs=1, space="PSUM") as ps_s, \
             tc.tile_pool(name="ps_o", bufs=1, space="PSUM") as ps_o:

            def emit_outproj(qc, lo=0, hi=8, yeng=None):
                # out-proj for a finished q-chunk; emitted during the NEXT
                # chunk's attention, shares the ot psum banks (WAR-ordered).
                # y DMAs ride the gpsimd SWDGE queue so the sync queue
                # stays clear for the latency-critical normalize DMAs.
                yeng = yeng or nc.gpsimd
                for qt_i in range(qc * 8 + lo, qc * 8 + hi):
                    ysb = yp.tile([128, D], BF16, tag="y", name=f"ysb{qt_i}")
                    for dc in range(2):
                        yps = ps_o.tile([128, 512], F32, tag=f"ot{dc}",
                                        name=f"yps{qt_i}{dc}")
                        for g in range(2):
                            nc.tensor.matmul(
                                yps[:],
                                otn_sb[g][:, qt_i * 128:(qt_i + 1) * 128],
                                wo_sb[g][:, dc * 512:(dc + 1) * 512],
                                start=(g == 0), stop=(g == 1),
                            )
                        nc.vector.tensor_copy(ysb[:, dc * 512:(dc + 1) * 512],
                                              yps[:])
                    yeng.dma_start(y[qt_i * 128:(qt_i + 1) * 128, :], ysb[:])

            pending = []
            prev_ot = None
            for qc in range(QCN):
                for j in range(2):          # head pair: heads 2j, 2j+1
                    opq = pending.pop() if (j == 1 and pending) else None
                    ot_ps = [ps_o.tile([HD + 1, QCW], F32, tag=f"ot{h2}", name=f"ot{qc}{j}{h2}")
                             for h2 in range(2)]
                    # separate scores/pt tiles PER H2: cross-engine
                    # deps resolve per tile, so the h0 and h1 pipelines
                    # decouple and the ACT exp stream runs back-to-back
                    s_ps = [ps_s.tile([128, QCW], F32, tag=f"s{h2}",
                                      name=f"s{qc}{j}{h2}")
                            for h2 in range(2)]
                    pts = {}

                    def emit_exp(kt, h2):
                        if (kt % 2, h2) not in pts:
                            pts[(kt % 2, h2)] = ptp.tile(
                                [128, QCW], ADT, tag=f"pt{h2}",
                                name=f"pt{kt % 2}{h2}")
                        nc.scalar.activation(
                            pts[(kt % 2, h2)][:],
                            s_ps[h2][:],
                            mybir.ActivationFunctionType.Exp,
                            scale=1.0 / SCALE,
                        )

                    def emit_scores(kt, h2):
                        for half in range(2):
                            nc.tensor.matmul(
                                s_ps[h2][:, half * 512:(half + 1) * 512],
                                kt_sb[j][h2 * 64:h2 * 64 + 64,
                                         kt * 128:(kt + 1) * 128],
                                qt_sb[j][h2 * 64:h2 * 64 + 64,
                                         qc * QCW + half * 512:
                                         qc * QCW + (half + 1) * 512],
                                start=True, stop=True,
                                tile_position=(h2 * 64, 0),
                            )

                    def emit_junk(n, base, targets=None):
                        # zero-weight accumulate: += 0 * qt. Real PE
                        # streaming (HAM stays warm), never changes the
                        # target psum (has_written bits stay intact).
                        tg = targets if targets is not None else ot_ps
                        for i in range(n):
                            src = (base * 512 + i * 512) % S
                            nc.tensor.matmul(
                                tg[i % 2][:, (i // 2 % 2) * 512:
                                          (i // 2 % 2) * 512 + 512],
                                zw[:, 0:HD + 1],
                                qt_sb[j][:, src:src + 512],
                                start=False, stop=False,
                                skip_group_check=True,
                            )

                    # ---- boundary: outproj block (or junk burst into the
                    # dead previous ot banks) laced with the kt=0 prologue
                    if opq is not None:
                        emit_outproj(opq, 0, 2)
                        emit_scores(0, 0)
                        emit_exp(0, 0)
                        emit_outproj(opq, 2, 8)
                        emit_scores(0, 1)
                        emit_exp(0, 1)
                    else:
                        if prev_ot is not None:
                            emit_junk(BOUNDARY_JUNK // 2, 0, prev_ot)
                        emit_scores(0, 0)
                        emit_exp(0, 0)
                        if prev_ot is not None:
                            emit_junk(BOUNDARY_JUNK // 2, 4, prev_ot)
                        emit_scores(0, 1)
                        emit_exp(0, 1)
                    for kt in range(QT):
                        if kt + 1 < QT:
                            pts.pop(((kt + 1) % 2, 0), None)
                            pts.pop(((kt + 1) % 2, 1), None)
                            emit_scores(kt + 1, 0)
                            emit_exp(kt + 1, 0)
                        # filler while ACT finishes exp(kt, h1)
                        if kt >= 1:
                            emit_junk(JUNK_PER_KT, kt)
                        if kt + 1 < QT:
                            emit_scores(kt + 1, 1)
                            emit_exp(kt + 1, 1)
                        for h2 in range(2):
                            h = 2 * j + h2
                            pt = pts[(kt % 2, h2)]
                            for half in range(2):
                                nc.tensor.matmul(
                                    ot_ps[h2][:, half * 512:(half + 1) * 512],
                                    v_sb[kt][:, h * 65:h * 65 + 65],
                                    pt[:, half * 512:(half + 1) * 512],
                                    start=(kt == 0), stop=(kt == QT - 1),
                                )
                        if kt == 0:
                            # ot bits now set; cover kt=0's stall window
                            emit_junk(JUNK_PER_KT, 0)
                    prev_ot = ot_ps
                    # ---- normalize: O.T * (1/sum) -> otn (f32r sbuf) ----
                    # reciprocal via DMA-transpose: [1,1024] row would be
                    # single-lane on DVE (6.5us); [128, 16] is ~free.
                    otr = [None, None]
                    for h2 in range(2):
                        otr[h2] = nrm.tile([HD + 1, QCW], F32, tag=f"otr{h2}",
                                           name=f"otr{qc}{j}{h2}")
                        nc.vector.tensor_copy(otr[h2][:], ot_ps[h2][:])
                    dT = nrm.tile([128, 16], F32, tag="dT")
                    for h2 in range(2):
                        nc.sync.dma_start(
                            dT[:, h2 * 8:(h2 + 1) * 8],
                            otr[h2][HD:HD + 1, :],
                        )
                    rT = nrm.tile([128, 16], F32, tag="rT")
                    nc.vector.reciprocal(rT[:], dT[:])
                    rcs = [nrm.tile([1, QCW], F32, tag=f"rc{h2}",
                                    name=f"rc{qc}{j}{h2}") for h2 in range(2)]
                    for h2 in range(2):
                        nc.sync.dma_start(
                            rcs[h2][:],
                            rT[:, h2 * 8:(h2 + 1) * 8],
                        )
                    for h2 in range(2):
                        sc = nrm.tile([HD, QCW], F32, tag=f"sc{h2}")
                        nc.gpsimd.partition_broadcast(sc[:], rcs[h2][:])
                        if h2 == 0:
                            nc.vector.tensor_mul(
                                otn_sb[j][0:HD, qc * QCW:(qc + 1) * QCW],
                                otr[0][0:HD, :], sc[:],
                            )
                        else:
                            stg = nrm.tile([HD, QCW], F32R, tag="stg")
                            nc.vector.tensor_mul(stg[:], otr[1][0:HD, :], sc[:])
                            nc.sync.dma_start(
                                otn_sb[j][HD:2 * HD, qc * QCW:(qc + 1) * QCW],
                                stg[:],
                            )
                pending.append(qc)
            # bridge the final normalize chain with filler so the HAM
            # stays warm into the last outproj block
            for i in range(FINAL_JUNK):
                src_off = (i * 512) % S
                nc.tensor.matmul(
                    prev_ot[i % 2][:, (i // 2 % 2) * 512:
                                   (i // 2 % 2) * 512 + 512],
                    zw[:, 0:HD + 1],
                    qt_sb[1][:, src_off:src_off + 512],
                    start=False, stop=False,
                    skip_group_check=True,
                )
            emit_outproj(pending.pop(), yeng=nc.sync)

    nc.compile()
    return nc


_NC_CACHE = None


def _get_nc():
    global _NC_CACHE
    if _NC_CACHE is None:
        _NC_CACHE = build_nc()
    return _NC_CACHE


def shard_inputs(query, key, value, Wq, bq, Wk, bk, Wv, bv, Wo, bo):
    """Build the 8 per-core input maps (host-side shard + transpose)."""
    import ml_dtypes
    f = np.float32
    bf = ml_dtypes.bfloat16
    in_maps = []
    for c in range(NCORES):
        b = c // 4
        g = c % 4
        hs = slice(g * DPC, (g + 1) * DPC)
        in_maps.append({
            "xq_t": np.ascontiguousarray(np.asarray(query[b], f).T.astype(bf)),
            "xk_t": np.ascontiguousarray(np.asarray(key[b], f).T.astype(bf)),
            "xv_t": np.ascontiguousarray(np.asarray(value[b], f).T.astype(bf)),
            "wq_t": np.ascontiguousarray(np.asarray(Wq[hs, :], f).T.astype(bf)),
            "wk_t": np.ascontiguousarray(np.asarray(Wk[hs, :], f).T.astype(bf)),
            "wv_t": np.ascontiguousarray(np.asarray(Wv[hs, :], f).T.astype(bf)),
            "wo_t": np.ascontiguousarray(np.asarray(Wo[:, hs], f).T),
            "bq": np.asarray(bq[hs], f).reshape(DPC, 1).copy(),
            "bk": np.asarray(bk[hs], f).reshape(DPC, 1).copy(),
            "bv": np.asarray(bv[hs], f).reshape(DPC, 1).copy(),
            "ident": np.eye(128, dtype=f),
        })
    return in_maps


def kernel(query, key, value, Wq, bq, Wk, bk, Wv, bv, Wo, bo, **run_kwargs):
    nc = _get_nc()
    in_maps = shard_inputs(query, key, value, Wq, bq, Wk, bk, Wv, bv, Wo, bo)
    res = run_bass_kernel_spmd(nc, in_maps, core_ids=list(range(NCORES)),
                               **run_kwargs)
    out = np.zeros((B, S, D), np.float32)
    for c in range(NCORES):
        out[c // 4] += np.asarray(res.results[c]["y"], np.float32)
    out += np.asarray(bo, np.float32)
    if run_kwargs:
        kernel.last_result = res
    return out


# revision 53
# speedup vs baseline: 1.0095x; 1.0095x over previous
"""Multi-head attention (B=2, S=2048, D=1024, H=16) on 8 NeuronCores.

Sharding: core c -> batch c//4, head-group c%4 (4 heads, 256 proj dims).
Per-core Bass/Tile kernel computes Q/K/V projections, transposed-scores
attention (k on partitions, softmax without max-subtraction), and a
partial output projection (row-parallel Wo). Host sums the 4 partials
per batch and adds bo.

v2 over the original baseline (which ran the attention phase at the
HAM-throttled 1.2GHz PE clock):
- x / Wq / Wk / Wv shipped bf16: halves input DMA, enables FWL.
- Scores psum is one [128, 2048] f32 tile; exp is issued per-h2
  [128, 1024] so scores(kt+1, h2) only WARs exp(kt, h2) and the ACT
  engine stays back-to-back saturated while scores matmuls hide.
- Zero-stationary junk matmuls (+= 0 * x into the live O.T psum
  accumulation) fill the PE stall windows: always-ready real PE
  streaming, keeps the HAM clock gate at 2.4GHz, costs no psum bank.
- Softmax denominator reciprocal via DMA-transpose to [128, 16]
  (was a 6.5us single-lane [1, 1024] DVE reciprocal, 52us total).
"""

import sys

sys.path.insert(0, "/opt/trn_rl_repo")

from contextlib import ExitStack

import numpy as np

import concourse.bacc as bacc
import concourse.mybir as mybir
import concourse.tile as tile
from concourse.bass_utils import run_bass_kernel_spmd

B = 2
S = 2048
D = 1024
H = 16
HD = 64
HPC = 4          # heads per core
DPC = HPC * HD   # 256 projection dims per core
NCORES = 8
SCALE = 8.0      # sqrt(HD)

F32 = mybir.dt.float32
F32R = mybir.dt.float32r
BF16 = mybir.dt.bfloat16
ADT = BF16   # attention operand dtype (qt/kt/v/pt)

DCH = D // 128   # 8 contraction chunks of 128
QT = S // 128    # 16 q-tiles / k-tiles of 128
QCN = 2          # attention q-chunks of 1024
QCW = 1024

JUNK_PER_KT = 2   # zero-weight filler matmuls per kt (HAM warmth)
BOUNDARY_JUNK = 12  # filler burst at group boundaries without outproj
PROJ_JUNK = 3
FINAL_JUNK = 40     # filler per projection d-chunk (DMA-paced phase)


def build_nc():
    nc = bacc.Bacc("TRN2", target_bir_lowering=False, debug=False, num_devices=NCORES)

    xq = nc.dram_tensor("xq_t", [D, S], BF16, kind="ExternalInput")
    xk = nc.dram_tensor("xk_t", [D, S], BF16, kind="ExternalInput")
    xv = nc.dram_tensor("xv_t", [D, S], BF16, kind="ExternalInput")
    wq = nc.dram_tensor("wq_t", [D, DPC], BF16, kind="ExternalInput")
    wk = nc.dram_tensor("wk_t", [D, DPC], BF16, kind="ExternalInput")
    wv = nc.dram_tensor("wv_t", [D, DPC], BF16, kind="ExternalInput")
    wo = nc.dram_tensor("wo_t", [DPC, D], F32R, kind="ExternalInput")
    bq = nc.dram_tensor("bq", [DPC, 1], F32, kind="ExternalInput")
    bk = nc.dram_tensor("bk", [DPC, 1], F32, kind="ExternalInput")
    bv = nc.dram_tensor("bv", [DPC, 1], F32, kind="ExternalInput")
    ident = nc.dram_tensor("ident", [128, 128], F32R, kind="ExternalInput")
    y = nc.dram_tensor("y", [S, D], BF16, kind="ExternalOutput")

    with tile.TileContext(nc) as tc, ExitStack() as ctx:
        const = ctx.enter_context(tc.tile_pool(name="const", bufs=1))
        xin = ctx.enter_context(tc.tile_pool(name="xin", bufs=5))
        qkv = ctx.enter_context(tc.tile_pool(name="qkv", bufs=1))
        ptp = ctx.enter_context(tc.tile_pool(name="ptp", bufs=2))
        nrm = ctx.enter_context(tc.tile_pool(name="nrm", bufs=2))
        yp = ctx.enter_context(tc.tile_pool(name="yp", bufs=3))

        # ---- constants / weights ----
        # tiny dummy exp first: preloads the ACT exp table off the
        # critical path
        dmy = const.tile([1, 16], F32, tag="dmy")
        nc.vector.memset(dmy[:], 0.0)
        dmy2 = const.tile([1, 16], F32, tag="dmy2")
        nc.scalar.activation(dmy2[:], dmy[:], mybir.ActivationFunctionType.Exp)

        id_sb = const.tile([128, 128], F32R, tag="id")

        # memset can't target f32r; stage in f32 and round via DVE copy
        onesv32 = const.tile([128, HPC], F32, tag="onesv32")
        nc.vector.memset(onesv32[:], 1.0)
        onesv = const.tile([128, HPC], ADT, tag="onesv")
        nc.vector.tensor_copy(onesv[:], onesv32[:])
        # zero-weight tile for HAM-filler matmuls (adds 0 to live psum)
        zw32 = const.tile([128, HD + 1], F32, tag="zw32")
        nc.vector.memset(zw32[:], 0.0)
        zw = const.tile([128, HD + 1], ADT, tag="zw")
        nc.vector.tensor_copy(zw[:], zw32[:])

        # weights: V first (V-projection runs first), then Q, K
        wv_sb = [const.tile([128, DPC], BF16, tag=f"wv{d}", name=f"wv{d}") for d in range(DCH)]
        wq_sb = [const.tile([128, DPC], BF16, tag=f"wq{d}", name=f"wq{d}") for d in range(DCH)]
        wk_sb = [const.tile([128, DPC], BF16, tag=f"wk{d}", name=f"wk{d}") for d in range(DCH)]
        bq_sb = [const.tile([128, 1], F32, tag=f"bq{hp}", name=f"bq{hp}") for hp in range(2)]
        bk_sb = [const.tile([128, 1], F32, tag=f"bk{hp}", name=f"bk{hp}") for hp in range(2)]
        bv_sb = [const.tile([128, 1], F32, tag=f"bv{hp}", name=f"bv{hp}") for hp in range(2)]
        # biases on the (idle) gpsimd SWDGE queue; weight matrices are
        # interleaved with the x chunk loads inside the proj loop so the
        # early hwdge queue bandwidth goes to the critical-path tensors
        for hp in range(2):
            nc.gpsimd.dma_start(bv_sb[hp][:], bv[hp * 128:(hp + 1) * 128, :])
            nc.gpsimd.dma_start(bq_sb[hp][:], bq[hp * 128:(hp + 1) * 128, :])
            nc.gpsimd.dma_start(bk_sb[hp][:], bk[hp * 128:(hp + 1) * 128, :])
        wo_sb = [const.tile([128, D], F32R, tag=f"wo{g}", name=f"wo{g}") for g in range(2)]
        w_dram = {"v": wv, "q": wq, "k": wk}
        w_sbs = {"v": wv_sb, "q": wq_sb, "k": wk_sb}

        # ---- V tiles (128, 4*65) with ones column, filled by PE transpose
        # of a V.T projection ----
        v_sb = [qkv.tile([128, HPC * (HD + 1)], ADT, tag=f"v{st}", name=f"v{st}") for st in range(QT)]
        for st in range(QT):
            v4 = v_sb[st][:].rearrange("p (h w) -> p h w", h=HPC)
            nc.vector.tensor_copy(
                v4[:, :, HD:HD + 1],
                onesv[:].rearrange("p (a b) -> p a b", b=1),
            )
        vt_sb = [qkv.tile([128, S], F32R, tag=f"vt{hp}", name=f"vtt{hp}") for hp in range(2)]

        # ---- Q.T / K.T projections: (d'=hp*128 partitions, s free) ----
        qt_sb = [qkv.tile([128, S], ADT, tag=f"qt{hp}", name=f"qtt{hp}") for hp in range(2)]
        kt_sb = [qkv.tile([128, S], ADT, tag=f"kt{hp}", name=f"ktt{hp}") for hp in range(2)]
        with tc.tile_pool(name="ps_p", bufs=1, space="PSUM") as ps_p:
            nc.scalar.dma_start(id_sb[:], ident[:])
            for g in range(2):
                nc.scalar.dma_start(wo_sb[g][:], wo[g * 128:(g + 1) * 128, :])
            tr_q = [(hp, st) for hp in range(2) for st in range(QT)]

            def emit_transposes(n):
                # V.T -> V transposes, interleaved into the q/k
                # projection passes (fills their DMA slack; the vt
                # source is complete once the v pass evacuated)
                for _ in range(min(n, len(tr_q))):
                    hp, st = tr_q.pop(0)
                    tp = ps_p.tile([128, 128], F32R, tag=f"pp{st % 8}",
                                   name=f"tp{hp}{st}")
                    nc.tensor.transpose(
                        tp[:],
                        vt_sb[hp][:, st * 128:(st + 1) * 128],
                        id_sb[:],
                    )
                    v4 = v_sb[st][:].rearrange("p (h w) -> p h w", h=HPC)
                    nc.vector.tensor_copy(
                        v4[:, 2 * hp:2 * hp + 2, 0:HD],
                        tp[:].rearrange("p (h w) -> p h w", h=2),
                    )

            for which, xin_dram, w_sb, b_sb, dst in (
                ("v", xv, wv_sb, bv_sb, vt_sb),
                ("q", xq, wq_sb, bq_sb, qt_sb),
                ("k", xk, wk_sb, bk_sb, kt_sb),
            ):
                accs = {}
                for hp in range(2):
                    for pc in range(4):
                        accs[(hp, pc)] = ps_p.tile([128, 512], F32, tag=f"pp{hp * 4 + pc}", name=f"pp_{which}{hp}{pc}")
                for d in range(DCH):
                    # weight chunk just ahead of its x chunk; x halves
                    # split across both hwdge queues for 2x bandwidth
                    nc.scalar.dma_start(w_sbs[which][d][:],
                                        w_dram[which][d * 128:(d + 1) * 128, :])
                    xt = xin.tile([128, S], BF16, tag="x")
                    nc.sync.dma_start(xt[:, 0:S // 2],
                                      xin_dram[d * 128:(d + 1) * 128, 0:S // 2])
                    nc.scalar.dma_start(xt[:, S // 2:S],
                                        xin_dram[d * 128:(d + 1) * 128, S // 2:S])
                    for hp in range(2):
                        for pc in range(4):
                            nc.tensor.matmul(
                                accs[(hp, pc)][:],
                                w_sb[d][:, hp * 128:(hp + 1) * 128],
                                xt[:, pc * 512:(pc + 1) * 512],
                                start=(d == 0), stop=(d == DCH - 1),
                            )
                    if which != "v" and not (which == "q" and d < 2):
                        emit_transposes(3)
                    if d >= 1:
                        # zero-weight filler vs the x-chunk DMA pacing:
                        # keeps the HAM clock warm through the proj phase
                        for i in range(PROJ_JUNK):
                            nc.tensor.matmul(
                                accs[(i % 2, i // 2 % 4)][0:HD + 1, 0:256],
                                zw[:],
                                w_sb[d][:],
                                start=False, stop=False,
                                skip_group_check=True,
                            )
                for hp in range(2):
                    for pc in range(4):
                        nc.vector.tensor_scalar_add(
                            dst[hp][:, pc * 512:(pc + 1) * 512],
                            accs[(hp, pc)][:],
                            b_sb[hp][:],
                        )
            emit_transposes(len(tr_q))

        # ---- attention + normalization, head-pairs packed on PE rows ----
        otn_sb = [qkv.tile([128, S], F32R, tag=f"otn{j}", name=f"otn{j}") for j in range(2)]
        with tc.tile_pool(name="ps_s", bufs=1, space="PSUM") as ps_s, \
             tc.tile_pool(name="ps_o", bufs=1, space="PSUM") as ps_o:

            def emit_outproj(qc, lo=0, hi=8, yeng=None):
                # out-proj for a finished q-chunk; emitted during the NEXT
                # chunk's attention, shares the ot psum banks (WAR-ordered).
                # y DMAs ride the gpsimd SWDGE queue so the sync queue
                # stays clear for the latency-critical normalize DMAs.
                yeng = yeng or nc.gpsimd
                for qt_i in range(qc * 8 + lo, qc * 8 + hi):
                    ysb = yp.tile([128, D], BF16, tag="y", name=f"ysb{qt_i}")
                    for dc in range(2):
                        yps = ps_o.tile([128, 512], F32, tag=f"ot{dc}",
                                        name=f"yps{qt_i}{dc}")
                        for g in range(2):
                            nc.tensor.matmul(
                                yps[:],
                                otn_sb[g][:, qt_i * 128:(qt_i + 1) * 128],
                                wo_sb[g][:, dc * 512:(dc + 1) * 512],
                                start=(g == 0), stop=(g == 1),
                            )
                        nc.vector.tensor_copy(ysb[:, dc * 512:(dc + 1) * 512],
                                              yps[:])
                    yeng.dma_start(y[qt_i * 128:(qt_i + 1) * 128, :], ysb[:])

            pending = []
            prev_ot = None
            for qc in range(QCN):
                for j in range(2):          # head pair: heads 2j, 2j+1
                    opq = pending.pop() if (j == 1 and pending) else None
                    ot_ps = [ps_o.tile([HD + 1, QCW], F32, tag=f"ot{h2}", name=f"ot{qc}{j}{h2}")
                             for h2 in range(2)]
                    # separate scores/pt tiles PER H2: cross-engine
                    # deps resolve per tile, so the h0 and h1 pipelines
                    # decouple and the ACT exp stream runs back-to-back
                    s_ps = [ps_s.tile([128, QCW], F32, tag=f"s{h2}",
                                      name=f"s{qc}{j}{h2}")
                            for h2 in range(2)]
                    pts = {}

                    def emit_exp(kt, h2):
                        if (kt % 2, h2) not in pts:
                            pts[(kt % 2, h2)] = ptp.tile(
                                [128, QCW], ADT, tag=f"pt{h2}",
                                name=f"pt{kt % 2}{h2}")
                        nc.scalar.activation(
                            pts[(kt % 2, h2)][:],
                            s_ps[h2][:],
                            mybir.ActivationFunctionType.Exp,
                            scale=1.0 / SCALE,
                        )

                    def emit_scores(kt, h2):
                        for half in range(2):
                            nc.tensor.matmul(
                                s_ps[h2][:, half * 512:(half + 1) * 512],
                                kt_sb[j][h2 * 64:h2 * 64 + 64,
                                         kt * 128:(kt + 1) * 128],
                                qt_sb[j][h2 * 64:h2 * 64 + 64,
                                         qc * QCW + half * 512:
                                         qc * QCW + (half + 1) * 512],
                                start=True, stop=True,
                                tile_position=(h2 * 64, 0),
                            )

                    def emit_junk(n, base, targets=None):
                        # zero-weight accumulate: += 0 * qt. Real PE
                        # streaming (HAM stays warm), never changes the
                        # target psum (has_written bits stay intact).
                        tg = targets if targets is not None else ot_ps
                        for i in range(n):
                            src = (base * 512 + i * 512) % S
                            nc.tensor.matmul(
                                tg[i % 2][:, (i // 2 % 2) * 512:
                                          (i // 2 % 2) * 512 + 512],
                                zw[:, 0:HD + 1],
                                qt_sb[j][:, src:src + 512],
                                start=False, stop=False,
                                skip_group_check=True,
                            )

                    # ---- boundary: outproj block (or junk burst into the
                    # dead previous ot banks) laced with the kt=0 prologue
                    if opq is not None:
                        emit_outproj(opq, 0, 2)
                        emit_scores(0, 0)
                        emit_exp(0, 0)
                        emit_outproj(opq, 2, 8)
                        emit_scores(0, 1)
                        emit_exp(0, 1)
                    else:
                        if prev_ot is not None:
                            emit_junk(BOUNDARY_JUNK // 2, 0, prev_ot)
                        emit_scores(0, 0)
                        emit_exp(0, 0)
                        if prev_ot is not None:
                            emit_junk(BOUNDARY_JUNK // 2, 4, prev_ot)
                        emit_scores(0, 1)
                        emit_exp(0, 1)
                    for kt in range(QT):
                        if kt + 1 < QT:
                            pts.pop(((kt + 1) % 2, 0), None)
                            pts.pop(((kt + 1) % 2, 1), None)
                            emit_scores(kt + 1, 0)
                            emit_exp(kt + 1, 0)
                        # filler while ACT finishes exp(kt, h1)
                        if kt >= 1:
                            emit_junk(JUNK_PER_KT, kt)
                        if kt + 1 < QT:
                            emit_scores(kt + 1, 1)
                            emit_exp(kt + 1, 1)
                        for h2 in range(2):
                            h = 2 * j + h2
                            pt = pts[(kt % 2, h2)]
                            for half in range(2):
                                nc.tensor.matmul(
                                    ot_ps[h2][:, half * 512:(half + 1) * 512],
                                    v_sb[kt][:, h * 65:h * 65 + 65],
                                    pt[:, half * 512:(half + 1) * 512],
                                    start=(kt == 0), stop=(kt == QT - 1),
                                )
                        if kt == 0:
                            # ot bits now set; cover kt=0's stall window
                            emit_junk(JUNK_PER_KT, 0)
                    prev_ot = ot_ps
                    # ---- normalize: O.T * (1/sum) -> otn (f32r sbuf) ----
                    # reciprocal via DMA-transpose: [1,1024] row would be
                    # single-lane on DVE (6.5us); [128, 16] is ~free.
                    otr = [None, None]
                    for h2 in range(2):
                        otr[h2] = nrm.tile([HD + 1, QCW], F32, tag=f"otr{h2}",
                                           name=f"otr{qc}{j}{h2}")
                        nc.vector.tensor_copy(otr[h2][:], ot_ps[h2][:])
                    dT = nrm.tile([128, 16], F32, tag="dT")
                    for h2 in range(2):
                        nc.sync.dma_start(
                            dT[:, h2 * 8:(h2 + 1) * 8],
                            otr[h2][HD:HD + 1, :],
                        )
                    rT = nrm.tile([128, 16], F32, tag="rT")
                    nc.vector.reciprocal(rT[:], dT[:])
                    rcs = [nrm.tile([1, QCW], F32, tag=f"rc{h2}",
                                    name=f"rc{qc}{j}{h2}") for h2 in range(2)]
                    for h2 in range(2):
                        nc.sync.dma_start(
                            rcs[h2][:],
                            rT[:, h2 * 8:(h2 + 1) * 8],
                        )
                    for h2 in range(2):
                        sc = nrm.tile([HD, QCW], F32, tag=f"sc{h2}")
                        nc.gpsimd.partition_broadcast(sc[:], rcs[h2][:])
                        if h2 == 0:
                            nc.vector.tensor_mul(
                                otn_sb[j][0:HD, qc * QCW:(qc + 1) * QCW],
                                otr[0][0:HD, :], sc[:],
                            )
                        else:
                            stg = nrm.tile([HD, QCW], F32R, tag="stg")
                            nc.vector.tensor_mul(stg[:], otr[1][0:HD, :], sc[:])
                            nc.sync.dma_start(
                                otn_sb[j][HD:2 * HD, qc * QCW:(qc + 1) * QCW],
                                stg[:],
                            )
                pending.append(qc)
            # bridge the final normalize chain with filler so the HAM
            # stays warm into the last outproj block
            for i in range(FINAL_JUNK):
                src_off = (i * 512) % S
                nc.tensor.matmul(
                    prev_ot[i % 2][:, (i // 2 % 2) * 512:
                                   (i // 2 % 2) * 512 + 512],
                    zw[:, 0:HD + 1],
                    qt_sb[1][:, src_off:src_off + 512],
                    start=False, stop=False,
                    skip_group_check=True,
                )
            emit_outproj(pending.pop(), yeng=nc.sync)

    nc.compile()
    return nc


_NC_CACHE = None


def _get_nc():
    global _NC_CACHE
    if _NC_CACHE is None:
        _NC_CACHE = build_nc()
    return _NC_CACHE


def shard_inputs(query, key, value, Wq, bq, Wk, bk, Wv, bv, Wo, bo):
    """Build the 8 per-core input maps (host-side shard + transpose)."""
    import ml_dtypes
    f = np.float32
    bf = ml_dtypes.bfloat16
    in_maps = []
    for c in range(NCORES):
        b = c // 4
        g = c % 4
        hs = slice(g * DPC, (g + 1) * DPC)
        in_maps.append({
            "xq_t": np.ascontiguousarray(np.asarray(query[b], f).T.astype(bf)),
            "xk_t": np.ascontiguousarray(np.asarray(key[b], f).T.astype(bf)),
            "xv_t": np.ascontiguousarray(np.asarray(value[b], f).T.astype(bf)),
            "wq_t": np.ascontiguousarray(np.asarray(Wq[hs, :], f).T.astype(bf)),
            "wk_t": np.ascontiguousarray(np.asarray(Wk[hs, :], f).T.astype(bf)),
            "wv_t": np.ascontiguousarray(np.asarray(Wv[hs, :], f).T.astype(bf)),
            "wo_t": np.ascontiguousarray(np.asarray(Wo[:, hs], f).T),
            "bq": np.asarray(bq[hs], f).reshape(DPC, 1).copy(),
            "bk": np.asarray(bk[hs], f).reshape(DPC, 1).copy(),
            "bv": np.asarray(bv[hs], f).reshape(DPC, 1).copy(),
            "ident": np.eye(128, dtype=f),
        })
    return in_maps


def kernel(query, key, value, Wq, bq, Wk, bk, Wv, bv, Wo, bo, **run_kwargs):
    nc = _get_nc()
    in_maps = shard_inputs(query, key, value, Wq, bq, Wk, bk, Wv, bv, Wo, bo)
    res = run_bass_kernel_spmd(nc, in_maps, core_ids=list(range(NCORES)),
                               **run_kwargs)
    out = np.zeros((B, S, D), np.float32)
    for c in range(NCORES):
        out[c // 4] += np.asarray(res.results[c]["y"], np.float32)
    out += np.asarray(bo, np.float32)
    if run_kwargs:
        kernel.last_result = res
    return out


# revision 54
# speedup vs baseline: 1.0124x; 1.0029x over previous
"""Multi-head attention (B=2, S=2048, D=1024, H=16) on 8 NeuronCores.

Sharding: core c -> batch c//4, head-group c%4 (4 heads, 256 proj dims).
Per-core Bass/Tile kernel computes Q/K/V projections, transposed-scores
attention (k on partitions, softmax without max-subtraction), and a
partial output projection (row-parallel Wo). Host sums the 4 partials
per batch and adds bo.

v2 over the original baseline (which ran the attention phase at the
HAM-throttled 1.2GHz PE clock; ~380us -> ~310us):
- x / Wq / Wk / Wv shipped bf16: halves input DMA; x chunks split
  across both HWDGE queues (sync + act); y output in bf16 via the
  gpsimd SWDGE queue so the sync queue stays clear for the
  latency-critical normalize transfers.
- SEPARATE scores/pt psum+sbuf tiles per h2: cross-engine deps are
  tile-granular, so the two heads' scores->exp->AV pipelines decouple
  and the ACT exp stream runs near back-to-back (1.3us/slot).
- Zero-stationary junk matmuls (+= 0 * x into live psum accumulations)
  fill PE stall windows everywhere (attention kt loop, projection
  DMA waits, group boundaries, the final normalize->outproj bridge):
  always-ready real PE streaming that keeps the HAM clock gate at
  2.4GHz without costing a psum bank.
- Softmax denominator reciprocal via DMA round-trip to [128, 16]
  (was a 6.5us single-lane [1, 1024] DVE reciprocal, 52us total).
- V.T->V transposes interleaved into the q/k projection passes'
  DMA slack instead of a serial (HAM-invisible) block.
"""

import sys

sys.path.insert(0, "/opt/trn_rl_repo")

from contextlib import ExitStack

import numpy as np

import concourse.bacc as bacc
import concourse.mybir as mybir
import concourse.tile as tile
from concourse.bass_utils import run_bass_kernel_spmd

B = 2
S = 2048
D = 1024
H = 16
HD = 64
HPC = 4          # heads per core
DPC = HPC * HD   # 256 projection dims per core
NCORES = 8
SCALE = 8.0      # sqrt(HD)

F32 = mybir.dt.float32
F32R = mybir.dt.float32r
BF16 = mybir.dt.bfloat16
ADT = BF16   # attention operand dtype (qt/kt/v/pt)

DCH = D // 128   # 8 contraction chunks of 128
QT = S // 128    # 16 q-tiles / k-tiles of 128
QCN = 2          # attention q-chunks of 1024
QCW = 1024

JUNK_PER_KT = 2   # zero-weight filler matmuls per kt (HAM warmth)
BOUNDARY_JUNK = 12  # filler burst at group boundaries without outproj
PROJ_JUNK = 3
FINAL_JUNK = 40     # filler per projection d-chunk (DMA-paced phase)


def build_nc():
    nc = bacc.Bacc("TRN2", target_bir_lowering=False, debug=False, num_devices=NCORES)

    xq = nc.dram_tensor("xq_t", [D, S], BF16, kind="ExternalInput")
    xk = nc.dram_tensor("xk_t", [D, S], BF16, kind="ExternalInput")
    xv = nc.dram_tensor("xv_t", [D, S], BF16, kind="ExternalInput")
    wq = nc.dram_tensor("wq_t", [D, DPC], BF16, kind="ExternalInput")
    wk = nc.dram_tensor("wk_t", [D, DPC], BF16, kind="ExternalInput")
    wv = nc.dram_tensor("wv_t", [D, DPC], BF16, kind="ExternalInput")
    wo = nc.dram_tensor("wo_t", [DPC, D], F32R, kind="ExternalInput")
    bq = nc.dram_tensor("bq", [DPC, 1], F32, kind="ExternalInput")
    bk = nc.dram_tensor("bk", [DPC, 1], F32, kind="ExternalInput")
    bv = nc.dram_tensor("bv", [DPC, 1], F32, kind="ExternalInput")
    ident = nc.dram_tensor("ident", [128, 128], F32R, kind="ExternalInput")
    y = nc.dram_tensor("y", [S, D], BF16, kind="ExternalOutput")

    with tile.TileContext(nc) as tc, ExitStack() as ctx:
        const = ctx.enter_context(tc.tile_pool(name="const", bufs=1))
        xin = ctx.enter_context(tc.tile_pool(name="xin", bufs=5))
        qkv = ctx.enter_context(tc.tile_pool(name="qkv", bufs=1))
        ptp = ctx.enter_context(tc.tile_pool(name="ptp", bufs=2))
        nrm = ctx.enter_context(tc.tile_pool(name="nrm", bufs=2))
        yp = ctx.enter_context(tc.tile_pool(name="yp", bufs=3))

        # ---- constants / weights ----
        # tiny dummy exp first: preloads the ACT exp table off the
        # critical path
        dmy = const.tile([1, 16], F32, tag="dmy")
        nc.vector.memset(dmy[:], 0.0)
        dmy2 = const.tile([1, 16], F32, tag="dmy2")
        nc.scalar.activation(dmy2[:], dmy[:], mybir.ActivationFunctionType.Exp)

        id_sb = const.tile([128, 128], F32R, tag="id")

        # memset can't target f32r; stage in f32 and round via DVE copy
        onesv32 = const.tile([128, HPC], F32, tag="onesv32")
        nc.vector.memset(onesv32[:], 1.0)
        onesv = const.tile([128, HPC], ADT, tag="onesv")
        nc.vector.tensor_copy(onesv[:], onesv32[:])
        # zero-weight tile for HAM-filler matmuls (adds 0 to live psum)
        zw32 = const.tile([128, HD + 1], F32, tag="zw32")
        nc.vector.memset(zw32[:], 0.0)
        zw = const.tile([128, HD + 1], ADT, tag="zw")
        nc.vector.tensor_copy(zw[:], zw32[:])

        # weights: V first (V-projection runs first), then Q, K
        wv_sb = [const.tile([128, DPC], BF16, tag=f"wv{d}", name=f"wv{d}") for d in range(DCH)]
        wq_sb = [const.tile([128, DPC], BF16, tag=f"wq{d}", name=f"wq{d}") for d in range(DCH)]
        wk_sb = [const.tile([128, DPC], BF16, tag=f"wk{d}", name=f"wk{d}") for d in range(DCH)]
        bq_sb = [const.tile([128, 1], F32, tag=f"bq{hp}", name=f"bq{hp}") for hp in range(2)]
        bk_sb = [const.tile([128, 1], F32, tag=f"bk{hp}", name=f"bk{hp}") for hp in range(2)]
        bv_sb = [const.tile([128, 1], F32, tag=f"bv{hp}", name=f"bv{hp}") for hp in range(2)]
        # biases on the (idle) gpsimd SWDGE queue; weight matrices are
        # interleaved with the x chunk loads inside the proj loop so the
        # early hwdge queue bandwidth goes to the critical-path tensors
        for hp in range(2):
            nc.gpsimd.dma_start(bv_sb[hp][:], bv[hp * 128:(hp + 1) * 128, :])
            nc.gpsimd.dma_start(bq_sb[hp][:], bq[hp * 128:(hp + 1) * 128, :])
            nc.gpsimd.dma_start(bk_sb[hp][:], bk[hp * 128:(hp + 1) * 128, :])
        wo_sb = [const.tile([128, D], F32R, tag=f"wo{g}", name=f"wo{g}") for g in range(2)]
        w_dram = {"v": wv, "q": wq, "k": wk}
        w_sbs = {"v": wv_sb, "q": wq_sb, "k": wk_sb}

        # ---- V tiles (128, 4*65) with ones column, filled by PE transpose
        # of a V.T projection ----
        v_sb = [qkv.tile([128, HPC * (HD + 1)], ADT, tag=f"v{st}", name=f"v{st}") for st in range(QT)]
        for st in range(QT):
            v4 = v_sb[st][:].rearrange("p (h w) -> p h w", h=HPC)
            nc.vector.tensor_copy(
                v4[:, :, HD:HD + 1],
                onesv[:].rearrange("p (a b) -> p a b", b=1),
            )
        vt_sb = [qkv.tile([128, S], F32R, tag=f"vt{hp}", name=f"vtt{hp}") for hp in range(2)]

        # ---- Q.T / K.T projections: (d'=hp*128 partitions, s free) ----
        qt_sb = [qkv.tile([128, S], ADT, tag=f"qt{hp}", name=f"qtt{hp}") for hp in range(2)]
        kt_sb = [qkv.tile([128, S], ADT, tag=f"kt{hp}", name=f"ktt{hp}") for hp in range(2)]
        with tc.tile_pool(name="ps_p", bufs=1, space="PSUM") as ps_p:
            nc.scalar.dma_start(id_sb[:], ident[:])
            for g in range(2):
                nc.scalar.dma_start(wo_sb[g][:], wo[g * 128:(g + 1) * 128, :])
            tr_q = [(hp, st) for hp in range(2) for st in range(QT)]

            def emit_transposes(n):
                # V.T -> V transposes, interleaved into the q/k
                # projection passes (fills their DMA slack; the vt
                # source is complete once the v pass evacuated)
                for _ in range(min(n, len(tr_q))):
                    hp, st = tr_q.pop(0)
                    tp = ps_p.tile([128, 128], F32R, tag=f"pp{st % 8}",
                                   name=f"tp{hp}{st}")
                    nc.tensor.transpose(
                        tp[:],
                        vt_sb[hp][:, st * 128:(st + 1) * 128],
                        id_sb[:],
                    )
                    v4 = v_sb[st][:].rearrange("p (h w) -> p h w", h=HPC)
                    nc.vector.tensor_copy(
                        v4[:, 2 * hp:2 * hp + 2, 0:HD],
                        tp[:].rearrange("p (h w) -> p h w", h=2),
                    )

            for which, xin_dram, w_sb, b_sb, dst in (
                ("v", xv, wv_sb, bv_sb, vt_sb),
                ("q", xq, wq_sb, bq_sb, qt_sb),
                ("k", xk, wk_sb, bk_sb, kt_sb),
            ):
                accs = {}
                for hp in range(2):
                    for pc in range(4):
                        accs[(hp, pc)] = ps_p.tile([128, 512], F32, tag=f"pp{hp * 4 + pc}", name=f"pp_{which}{hp}{pc}")
                for d in range(DCH):
                    # weight chunk just ahead of its x chunk; x halves
                    # split across both hwdge queues for 2x bandwidth
                    nc.scalar.dma_start(w_sbs[which][d][:],
                                        w_dram[which][d * 128:(d + 1) * 128, :])
                    xt = xin.tile([128, S], BF16, tag="x")
                    nc.sync.dma_start(xt[:, 0:S // 2],
                                      xin_dram[d * 128:(d + 1) * 128, 0:S // 2])
                    nc.scalar.dma_start(xt[:, S // 2:S],
                                        xin_dram[d * 128:(d + 1) * 128, S // 2:S])
                    for hp in range(2):
                        for pc in range(4):
                            nc.tensor.matmul(
                                accs[(hp, pc)][:],
                                w_sb[d][:, hp * 128:(hp + 1) * 128],
                                xt[:, pc * 512:(pc + 1) * 512],
                                start=(d == 0), stop=(d == DCH - 1),
                            )
                    if which != "v" and not (which == "q" and d < 2):
                        emit_transposes(3)
                    if d >= 1:
                        # zero-weight filler vs the x-chunk DMA pacing:
                        # keeps the HAM clock warm through the proj phase
                        for i in range(PROJ_JUNK):
                            nc.tensor.matmul(
                                accs[(i % 2, i // 2 % 4)][0:HD + 1, 0:256],
                                zw[:],
                                w_sb[d][:],
                                start=False, stop=False,
                                skip_group_check=True,
                            )
                for hp in range(2):
                    for pc in range(4):
                        nc.vector.tensor_scalar_add(
                            dst[hp][:, pc * 512:(pc + 1) * 512],
                            accs[(hp, pc)][:],
                            b_sb[hp][:],
                        )
            emit_transposes(len(tr_q))

        # ---- attention + normalization, head-pairs packed on PE rows ----
        otn_sb = [qkv.tile([128, S], F32R, tag=f"otn{j}", name=f"otn{j}") for j in range(2)]
        with tc.tile_pool(name="ps_s", bufs=1, space="PSUM") as ps_s, \
             tc.tile_pool(name="ps_o", bufs=1, space="PSUM") as ps_o:

            def emit_outproj(qc, lo=0, hi=8, yeng=None):
                # out-proj for a finished q-chunk; emitted during the NEXT
                # chunk's attention, shares the ot psum banks (WAR-ordered).
                # y DMAs ride the gpsimd SWDGE queue so the sync queue
                # stays clear for the latency-critical normalize DMAs.
                yeng = yeng or nc.gpsimd
                for qt_i in range(qc * 8 + lo, qc * 8 + hi):
                    ysb = yp.tile([128, D], BF16, tag="y", name=f"ysb{qt_i}")
                    for dc in range(2):
                        yps = ps_o.tile([128, 512], F32, tag=f"ot{dc}",
                                        name=f"yps{qt_i}{dc}")
                        for g in range(2):
                            nc.tensor.matmul(
                                yps[:],
                                otn_sb[g][:, qt_i * 128:(qt_i + 1) * 128],
                                wo_sb[g][:, dc * 512:(dc + 1) * 512],
                                start=(g == 0), stop=(g == 1),
                            )
                        nc.vector.tensor_copy(ysb[:, dc * 512:(dc + 1) * 512],
                                              yps[:])
                    yeng.dma_start(y[qt_i * 128:(qt_i + 1) * 128, :], ysb[:])

            pending = []
            prev_ot = None
            for qc in range(QCN):
                for j in range(2):          # head pair: heads 2j, 2j+1
                    opq = pending.pop() if (j == 1 and pending) else None
                    ot_ps = [ps_o.tile([HD + 1, QCW], F32, tag=f"ot{h2}", name=f"ot{qc}{j}{h2}")
                             for h2 in range(2)]
                    # separate scores/pt tiles PER H2: cross-engine
                    # deps resolve per tile, so the h0 and h1 pipelines
                    # decouple and the ACT exp stream runs back-to-back
                    s_ps = [ps_s.tile([128, QCW], F32, tag=f"s{h2}",
                                      name=f"s{qc}{j}{h2}")
                            for h2 in range(2)]
                    pts = {}

                    def emit_exp(kt, h2):
                        if (kt % 2, h2) not in pts:
                            pts[(kt % 2, h2)] = ptp.tile(
                                [128, QCW], ADT, tag=f"pt{h2}",
                                name=f"pt{kt % 2}{h2}")
                        nc.scalar.activation(
                            pts[(kt % 2, h2)][:],
                            s_ps[h2][:],
                            mybir.ActivationFunctionType.Exp,
                            scale=1.0 / SCALE,
                        )

                    def emit_scores(kt, h2):
                        for half in range(2):
                            nc.tensor.matmul(
                                s_ps[h2][:, half * 512:(half + 1) * 512],
                                kt_sb[j][h2 * 64:h2 * 64 + 64,
                                         kt * 128:(kt + 1) * 128],
                                qt_sb[j][h2 * 64:h2 * 64 + 64,
                                         qc * QCW + half * 512:
                                         qc * QCW + (half + 1) * 512],
                                start=True, stop=True,
                                tile_position=(h2 * 64, 0),
                            )

                    def emit_junk(n, base, targets=None):
                        # zero-weight accumulate: += 0 * qt. Real PE
                        # streaming (HAM stays warm), never changes the
                        # target psum (has_written bits stay intact).
                        tg = targets if targets is not None else ot_ps
                        for i in range(n):
                            src = (base * 512 + i * 512) % S
                            nc.tensor.matmul(
                                tg[i % 2][:, (i // 2 % 2) * 512:
                                          (i // 2 % 2) * 512 + 512],
                                zw[:, 0:HD + 1],
                                qt_sb[j][:, src:src + 512],
                                start=False, stop=False,
                                skip_group_check=True,
                            )

                    # ---- boundary: outproj block (or junk burst into the
                    # dead previous ot banks) laced with the kt=0 prologue
                    if opq is not None:
                        emit_outproj(opq, 0, 2)
                        emit_scores(0, 0)
                        emit_exp(0, 0)
                        emit_outproj(opq, 2, 8)
                        emit_scores(0, 1)
                        emit_exp(0, 1)
                    else:
                        if prev_ot is not None:
                            emit_junk(BOUNDARY_JUNK // 2, 0, prev_ot)
                        emit_scores(0, 0)
                        emit_exp(0, 0)
                        if prev_ot is not None:
                            emit_junk(BOUNDARY_JUNK // 2, 4, prev_ot)
                        emit_scores(0, 1)
                        emit_exp(0, 1)
                    for kt in range(QT):
                        if kt + 1 < QT:
                            pts.pop(((kt + 1) % 2, 0), None)
                            pts.pop(((kt + 1) % 2, 1), None)
                            emit_scores(kt + 1, 0)
                            emit_exp(kt + 1, 0)
                        # filler while ACT finishes exp(kt, h1)
                        if kt >= 1:
                            emit_junk(JUNK_PER_KT, kt)
                        if kt + 1 < QT:
                            emit_scores(kt + 1, 1)
                            emit_exp(kt + 1, 1)
                        for h2 in range(2):
                            h = 2 * j + h2
                            pt = pts[(kt % 2, h2)]
                            for half in range(2):
                                nc.tensor.matmul(
                                    ot_ps[h2][:, half * 512:(half + 1) * 512],
                                    v_sb[kt][:, h * 65:h * 65 + 65],
                                    pt[:, half * 512:(half + 1) * 512],
                                    start=(kt == 0), stop=(kt == QT - 1),
                                )
                        if kt == 0:
                            # ot bits now set; cover kt=0's stall window
                            emit_junk(JUNK_PER_KT, 0)
                    prev_ot = ot_ps
                    # ---- normalize: O.T * (1/sum) -> otn (f32r sbuf) ----
                    # reciprocal via DMA-transpose: [1,1024] row would be
                    # single-lane on DVE (6.5us); [128, 16] is ~free.
                    otr = [None, None]
                    for h2 in range(2):
                        otr[h2] = nrm.tile([HD + 1, QCW], F32, tag=f"otr{h2}",
                                           name=f"otr{qc}{j}{h2}")
                        nc.vector.tensor_copy(otr[h2][:], ot_ps[h2][:])
                    dT = nrm.tile([128, 16], F32, tag="dT")
                    for h2 in range(2):
                        nc.sync.dma_start(
                            dT[:, h2 * 8:(h2 + 1) * 8],
                            otr[h2][HD:HD + 1, :],
                        )
                    rT = nrm.tile([128, 16], F32, tag="rT")
                    nc.vector.reciprocal(rT[:], dT[:])
                    rcs = [nrm.tile([1, QCW], F32, tag=f"rc{h2}",
                                    name=f"rc{qc}{j}{h2}") for h2 in range(2)]
                    for h2 in range(2):
                        nc.sync.dma_start(
                            rcs[h2][:],
                            rT[:, h2 * 8:(h2 + 1) * 8],
                        )
                    for h2 in range(2):
                        sc = nrm.tile([HD, QCW], F32, tag=f"sc{h2}")
                        nc.gpsimd.partition_broadcast(sc[:], rcs[h2][:])
                        if h2 == 0:
                            nc.vector.tensor_mul(
                                otn_sb[j][0:HD, qc * QCW:(qc + 1) * QCW],
                                otr[0][0:HD, :], sc[:],
                            )
                        else:
                            stg = nrm.tile([HD, QCW], F32R, tag="stg")
                            nc.vector.tensor_mul(stg[:], otr[1][0:HD, :], sc[:])
                            nc.sync.dma_start(
                                otn_sb[j][HD:2 * HD, qc * QCW:(qc + 1) * QCW],
                                stg[:],
                            )
                pending.append(qc)
            # bridge the final normalize chain with filler so the HAM
            # stays warm into the last outproj block
            for i in range(FINAL_JUNK):
                src_off = (i * 512) % S
                nc.tensor.matmul(
                    prev_ot[i % 2][:, (i // 2 % 2) * 512:
                                   (i // 2 % 2) * 512 + 512],
                    zw[:, 0:HD + 1],
                    qt_sb[1][:, src_off:src_off + 512],
                    start=False, stop=False,
                    skip_group_check=True,
                )
            emit_outproj(pending.pop(), yeng=nc.sync)

    nc.compile()
    return nc


_NC_CACHE = None


def _get_nc():
    global _NC_CACHE
    if _NC_CACHE is None:
        _NC_CACHE = build_nc()
    return _NC_CACHE


def shard_inputs(query, key, value, Wq, bq, Wk, bk, Wv, bv, Wo, bo):
    """Build the 8 per-core input maps (host-side shard + transpose)."""
    import ml_dtypes
    f = np.float32
    bf = ml_dtypes.bfloat16
    in_maps = []
    for c in range(NCORES):
        b = c // 4
        g = c % 4
        hs = slice(g * DPC, (g + 1) * DPC)
        in_maps.append({
            "xq_t": np.ascontiguousarray(np.asarray(query[b], f).T.astype(bf)),
            "xk_t": np.ascontiguousarray(np.asarray(key[b], f).T.astype(bf)),
            "xv_t": np.ascontiguousarray(np.asarray(value[b], f).T.astype(bf)),
            "wq_t": np.ascontiguousarray(np.asarray(Wq[hs, :], f).T.astype(bf)),
            "wk_t": np.ascontiguousarray(np.asarray(Wk[hs, :], f).T.astype(bf)),
            "wv_t": np.ascontiguousarray(np.asarray(Wv[hs, :], f).T.astype(bf)),
            "wo_t": np.ascontiguousarray(np.asarray(Wo[:, hs], f).T),
            "bq": np.asarray(bq[hs], f).reshape(DPC, 1).copy(),
            "bk": np.asarray(bk[hs], f).reshape(DPC, 1).copy(),
            "bv": np.asarray(bv[hs], f).reshape(DPC, 1).copy(),
            "ident": np.eye(128, dtype=f),
        })
    return in_maps


def kernel(query, key, value, Wq, bq, Wk, bk, Wv, bv, Wo, bo, **run_kwargs):
    nc = _get_nc()
    in_maps = shard_inputs(query, key, value, Wq, bq, Wk, bk, Wv, bv, Wo, bo)
    res = run_bass_kernel_spmd(nc, in_maps, core_ids=list(range(NCORES)),
                               **run_kwargs)
    out = np.zeros((B, S, D), np.float32)
    for c in range(NCORES):
        out[c // 4] += np.asarray(res.results[c]["y"], np.float32)
    out += np.asarray(bo, np.float32)
    if run_kwargs:
        kernel.last_result = res
    return out
